# revision 63
# baseline (speedup 1.0000x reference)
"""Complex multihead attention Trainium2 kernel (no collectives).

Sharding: core c = (batch b=c//2, query-half qh=c%2). Each core computes
K/V projections for the full sequence (all 8 heads), Q projection for
its 512 queries, attention for all heads over its queries, and a fully
local residual + LayerNorm over the full 512 dims. No cross-core
communication (collectives in this runtime cost 30-50us of unhidable
tail latency, more than the duplicated K/V projection work).

Compute dtype: fp8e4 (e4m3) operands on the PE with DoubleRow perf mode
(two 128-deep contraction tiles per matmul instruction) for every
contraction >= 256: the Q/K/V projections (contraction 1024 = 4 pairs),
the AV application and the softmax-denominator rsum (contraction over
S=1024 keys = 4 kt pairs). Scores (contraction 128 = r/i x dh) stay
plain-mode fp8. f32 PSUM accumulation throughout; the residual path
stays bf16 and the LayerNorm f32, so fp8 noise in the attention path is
diluted ~20x by the residual before the final normalization.

Inputs arrive as interleaved fp8 [rows, D, 2]; PE-transposes (1 cyc/row)
build the stacked-transposed layouts (XBAR dma_start_transpose moves
data in 256B packets here and is ~4x slower than plain DMA). Weights are
loaded compact (pattern1 only) and pattern2 = (-Wi^T | Wr^T) is derived
on the otherwise-idle DVE. Qv2 is derived from Qv1 by a partition-swap
DMA + negate instead of a second projection matmul.

Natural-tile loads alternate the sync/scalar hwdge queues.
DMA queue discipline (queues are in-order; a dependency-stalled DMA at
the head blocks everything behind it): sync carries the natural-tile x
loads + output stores, gpsimd (software DGE, async transfers) carries
weights/broadcasts/residual and the tiny per-job rsb copies, scalar
carries only the Qv2 partition-swap.

Phase B runs 8 jobs (one per head, 512 queries), software-pipelined
with depth 2: scores of jobs j+1/j+2 are issued before AV of job j so
the Scalar-engine exp chain overlaps the PE's AV/rsum work (the PE
queue is in-order; without this the PE head-of-line blocks on exp).
The first two jobs' scores are issued before the V projection so exp
hides behind V matmuls. Residual add + deferred V-bias + LayerNorm
partial stats (bn_stats on 64-dim head blocks) run per job, overlapped;
the tail only aggregates (bn_aggr), normalizes and stores. The LN
affine (gamma/beta) ops are compiled out when gamma==1 and beta==0.

On-chip layouts (per core, S=1024 keys, TQ=512 queries):
  xkT/xvT [128, 8ch*S] fp8, xqT [128, 8ch*TQ]: ch<4 -> Xr^T d-block,
        ch>=4 -> Xi^T (stacked transposed inputs).
  wq/wk [128, 4ds*2048] fp8: per d_sub, pattern1 (Wr^T|Wi^T)
        head-paired cols 0:1024, pattern2 (-Wi^T|Wr^T) 1024:2048.
  wv    [128, 4ds*2048] fp8: per d_sub, lo (Wr^T|Wi^T) plain 0:1024,
        hi (-Wi^T|Wr^T) 1024:2048.
  K_stk [128, 8h*S] fp8: rows 0:64 Kr^T, 64:128 Ki^T per head (+bias).
  Qv1   [128, 8h*TQ] fp8: [Qr^T; -Qi^T] per head (bias folded).
  Qv2   [128, 8h*TQ] fp8: [Qi^T; Qr^T].
  V_all [128, 8kt*1024] fp8: per k-tile, head-paired [Vr_h 64|Vi_h 64].
  scores psum [128 k, 1024] f32 per (job, kt): 0:512 Sr^T, 512: Si^T
  E = exp(0.125*S^T) sbuf fp8 [128, 2048] per kt PAIR (DoubleRow rhs)
  P12 psum [128, 1024] f32 = sum_kp V^T [Er|Ei] -> [ErVr;ErVi|EiVr;EiVi]
  rp psum [1, 1024] f32 = ones^T [Er | Ei] (softmax denominators)
  OUT_int [128, 4qs*1024] f32: (d,c)-interleaved full-D rows=q
  stat_raw [128, 4qs*2c*8h*6] f32: per-job bn_stats partials.
"""

import math
from contextlib import ExitStack

import numpy as np
import ml_dtypes

import concourse.bass as bass
import concourse.tile as tile
from concourse import bacc, mybir
from concourse.bass_utils import run_bass_kernel_spmd
from concourse.masks import make_identity

B, S, D, H = 4, 1024, 512, 8
DH = D // H  # 64
TQ = S // 2  # 512 queries per core
NCORES = 8
EPS = 1e-5
F32 = mybir.dt.float32
F32R = mybir.dt.float32r
BF16 = mybir.dt.bfloat16
F8 = mybir.dt.float8e4
DR = mybir.MatmulPerfMode.DoubleRow

NKT = S // 128  # 8 key tiles
NQS = TQ // 128  # 4 query subtiles
NCH = 8  # contraction chunks (2*D/128)
NPR = 4  # DoubleRow contraction pairs
SCALE = 1.0 / math.sqrt(DH)


def build_nc(ln_affine: bool = True) -> bass.Bass:
    nc = bacc.Bacc(None, target_bir_lowering=False, debug=False)

    # x inputs arrive pre-transposed from the host into the stacked
    # layout (ch = c*4 + d_block, partition = d-within-block): no on-chip
    # transposes, and the layouts are sliced so each compute stage's DMA
    # lands just ahead of its matmuls (xk by 512-query half, xv by
    # 128-key tile).
    xq_d = nc.declare_dram_parameter("xqt", [128, NCH * TQ], F8, isOutput=False)
    xk_d = nc.declare_dram_parameter("xkt", [128, 2, NCH, 512], F8, isOutput=False)
    xv_d = nc.declare_dram_parameter("xvt", [128, NKT, NCH, 128], F8, isOutput=False)
    rq_r = nc.declare_dram_parameter("rq_r", [128, NQS, D], BF16, isOutput=False)
    rq_i = nc.declare_dram_parameter("rq_i", [128, NQS, D], BF16, isOutput=False)
    wq_d = nc.declare_dram_parameter("wq", [D, 1024], F8, isOutput=False)
    wk_d = nc.declare_dram_parameter("wk", [D, 1024], F8, isOutput=False)
    wv_d = nc.declare_dram_parameter("wv", [D, 1024], F8, isOutput=False)
    bq_d = nc.declare_dram_parameter("bq_stk", [128, H], F32, isOutput=False)
    bk_d = nc.declare_dram_parameter("bk_stk", [128, H], F32, isOutput=False)
    bv_d = nc.declare_dram_parameter("bv_int", [1, 2 * D], F32, isOutput=False)
    gam_d = nc.declare_dram_parameter("gam_int", [1, 2 * D], F32, isOutput=False)
    bet_d = nc.declare_dram_parameter("bet_int", [1, 2 * D], F32, isOutput=False)
    # output stored bf16 (host upconverts): halves the 2MB store and
    # the tail's last-store latency; adds <=0.4% quantization on the
    # normalized output, well inside the error budget.
    out_d = nc.declare_dram_parameter("out", [TQ, D, 2], BF16, isOutput=True)

    with tile.TileContext(nc) as tc, ExitStack() as ctx:
        consts = ctx.enter_context(tc.tile_pool(name="consts", bufs=1))
        attn_in = ctx.enter_context(tc.tile_pool(name="attn_in", bufs=1))

        ident_f = consts.tile([128, 128], F32)
        ident_b = consts.tile([128, 128], BF16)
        ident_r = consts.tile([2, 2], F32R)
        # dual-fp8 ldweights requires the pair dim's step % 16 == 0:
        # keep the two ones columns 16 apart.
        ones_f = consts.tile([128, 32], F32)
        nc.vector.memset(ones_f, 1.0)
        ones_8 = consts.tile([128, 32], F8)
        nc.vector.tensor_copy(out=ones_8, in_=ones_f)
        eps_t = consts.tile([128, 1], F32)
        nc.vector.memset(eps_t, EPS)
        # per-partition sign vector for the Q evacuation on ACT:
        # rows 0:64 -> +1 (real part), rows 64:128 -> -1 (negated imag)
        sign_t = consts.tile([128, 1], F32)
        nc.vector.memset(sign_t, 1.0)
        nc.vector.memset(sign_t[DH:128, :], -1.0)

        # ---- DMA queue discipline ----
        # sync (hwdge): xk/xq/xv stacked loads + even stores; scalar
        # (hwdge): weights, Qv2 swap, residual, odd stores; gpsimd
        # (swdge): broadcasts + per-job rsb copies.
        st_v = ExitStack()
        st_q = ExitStack()
        st_k = ExitStack()
        xtv_pool = st_v.enter_context(tc.tile_pool(name="xtv", bufs=1))
        wv_pool = st_v.enter_context(tc.tile_pool(name="wvp", bufs=1))
        xtq_pool = st_q.enter_context(tc.tile_pool(name="xtq", bufs=1))
        wq_pool = st_q.enter_context(tc.tile_pool(name="wqp", bufs=1))
        xtk_pool = st_k.enter_context(tc.tile_pool(name="xtk", bufs=1))
        wk_pool = st_k.enter_context(tc.tile_pool(name="wkp", bufs=1))

        # PSUM pool lifetimes are LIFO per space; 16KB/partition budget:
        #   K/Q phase:   sc 8K + proj 4K          = 12K
        #   V stage:     sc 8K + vp 8K            = 16K
        #   AV phase:    sc 8K + p12 4K + r 4K    = 16K
        sc_psum = ctx.enter_context(
            tc.tile_pool(name="sc_ps", bufs=2, space="PSUM")
        )
        ps_p = ExitStack()
        proj_psum = ps_p.enter_context(
            tc.tile_pool(name="proj_ps", bufs=2, space="PSUM")
        )

        def load_w(w_pool, w_dram, tag, headpaired, eng=None):
            """One 512KB DMA (pattern1, ds-strided in SBUF); derive
            pattern2 = (-Wi | Wr) on DVE."""
            w = w_pool.tile([128, 4 * 2048], F8, tag=tag)
            ap0 = w_dram[:]
            (eng or nc.gpsimd).dma_start(
                out=bass.AP(
                    tensor=w.tensor, offset=w.offset,
                    ap=[w.ap[0], [2048, 4], [1, 1024]],
                ),
                in_=bass.AP(
                    tensor=ap0.tensor, offset=0,
                    ap=[[1024, 128], [128 * 1024, 4], [1, 1024]],
                ),
            )
            blk = 64 if headpaired else 512
            nb = 1024 // (2 * blk)
            def ap_of(col0):
                return bass.AP(
                    tensor=w.tensor,
                    offset=w.offset + col0,
                    ap=[w.ap[0], [2048, 4], [2 * blk, nb], [1, blk]],
                )
            nc.vector.tensor_scalar_mul(
                out=ap_of(1024), in0=ap_of(blk), scalar1=-1.0
            )
            nc.vector.tensor_copy(out=ap_of(1024 + blk), in_=ap_of(0))
            return w

        bq_stk = consts.tile([128, H], F32)
        nc.gpsimd.dma_start(out=bq_stk, in_=bq_d[:])
        bk_stk = consts.tile([128, H], F32)
        nc.gpsimd.dma_start(out=bk_stk, in_=bk_d[:])

        # attention-phase operand tensors
        K_stk = attn_in.tile([128, H * S], F8)
        V_pairs = [
            attn_in.tile([128, 2 * 1024], F8, name=f"vpair{i}",
                         tag=f"vpair{i}")
            for i in range(NKT // 2)
        ]
        Qv1 = attn_in.tile([128, H * TQ], F8)
        Qv2 = attn_in.tile([128, H * TQ], F8)

        def w_pair(w, col0, pr, ncol=128):
            """DoubleRow operand: chunk pair (2*pr, 2*pr+1) = adjacent
            d_subs of pattern pr//2, columns col0:col0+ncol."""
            base = (pr % 2) * 2 * 2048 + (pr // 2) * 1024 + col0
            return bass.AP(
                tensor=w.tensor,
                offset=w.offset + base,
                ap=[w.ap[0], [2048, 2], [1, ncol]],
            )

        # ---------------- phase A: projections --------------------------
        # -- K stage, split by 512-key halves: each half's matmuls start
        # as soon as its 512KB DMA slice lands. wk rides the scalar hwdge
        # queue (it gates the first matmul; the swdge gpsimd queue is
        # ~4x slower to first byte).
        wk = load_w(wk_pool, wk_d, "wk", True, eng=nc.scalar)
        # SBUF layout = host layout [tch][ch][512]: the DMA stays fully
        # contiguous (scattered-destination DMAs run at <1/4 bandwidth);
        # the DoubleRow pair AP just uses stride 512 instead of S.
        xkT = xtk_pool.tile([128, 2 * NCH * 512], F8, tag="xkT")

        def load_xk_half(tch):
            if tch == 0:
                # split the first (critical) half across two queues
                nc.sync.dma_start(
                    out=xkT[:, 0:2048], in_=xk_d[:, 0, 0:4],
                )
                nc.gpsimd.dma_start(
                    out=xkT[:, 2048:4096], in_=xk_d[:, 0, 4:8],
                )
            else:
                nc.sync.dma_start(
                    out=xkT[:, tch * 4096 : (tch + 1) * 4096],
                    in_=xk_d[:, tch],
                )

        def xk_pair(pr, tch):
            return bass.AP(
                tensor=xkT.tensor,
                offset=xkT.offset + tch * 4096 + (2 * pr) * 512,
                ap=[xkT.ap[0], [512, 2], [1, 512]],
            )

        def k_half(tch):
            for h in range(H):
                ps = proj_psum.tile([128, 512], F32, tag="proj")
                for pr in range(NPR):
                    nc.tensor.matmul(
                        ps,
                        w_pair(wk, 128 * h, pr),
                        xk_pair(pr, tch),
                        start=(pr == 0),
                        stop=(pr == NPR - 1),
                        perf_mode=DR,
                    )
                nc.scalar.activation(
                    out=K_stk[:, h * S + tch * 512 : h * S + (tch + 1) * 512],
                    in_=ps,
                    func=mybir.ActivationFunctionType.Identity,
                    bias=bk_stk[:, h : h + 1],
                )

        load_xk_half(0)
        k_half(0)
        load_xk_half(1)
        # identity tables (for the P12t/r transposes much later) are
        # built here so their DMAs queue behind the critical xk loads
        make_identity(nc, ident_f)
        nc.vector.tensor_copy(out=ident_b, in_=ident_f)
        nc.vector.tensor_copy(out=ident_r, in_=ident_f[0:2, 0:2])
        k_half(1)
        st_k.close()  # free xkT + wk SBUF

        # -- Q stage (8 heads, TQ queries; Qv1 only; Qv2 derived).
        # Evacuation fused on ACT: out = ps*sign + bias with per-partition
        # sign (+1 top / -1 bottom) and sign-folded bias from the host.
        wq = load_w(wq_pool, wq_d, "wq", True, eng=nc.scalar)
        xqT = xtq_pool.tile([128, NCH * TQ], F8, tag="xqT")
        nc.sync.dma_start(out=xqT, in_=xq_d[:])
        for h in range(H):
            ps = proj_psum.tile([128, TQ], F32, tag="proj")
            for pr in range(NPR):
                nc.tensor.matmul(
                    ps,
                    w_pair(wq, 128 * h, pr),
                    bass.AP(
                        tensor=xqT.tensor,
                        offset=xqT.offset + (2 * pr) * 512,
                        ap=[xqT.ap[0], [512, 2], [1, 512]],
                    ),
                    start=(pr == 0),
                    stop=(pr == NPR - 1),
                    perf_mode=DR,
                )
            nc.scalar.activation(
                out=Qv1[:, h * TQ : (h + 1) * TQ],
                in_=ps,
                func=mybir.ActivationFunctionType.Identity,
                bias=bq_stk[:, h : h + 1],
                scale=sign_t,
            )
        # Qv2 = [Qi; Qr] = [-Qv1_bot ; Qv1_top]
        nc.scalar.dma_start(out=Qv2[DH:128, :], in_=Qv1[0:DH, :])
        nc.scalar.dma_start(out=Qv2[0:DH, :], in_=Qv1[DH:128, :])
        nc.vector.tensor_scalar_mul(
            out=Qv2[0:DH, :], in0=Qv2[0:DH, :], scalar1=-1.0
        )
        ps_p.close()  # free projection psum banks
        st_q.close()  # free xqT + wq SBUF

        # ---------------- phase B pools (scores pool opens early) --------
        phase_b = ExitStack()
        outp = phase_b.enter_context(tc.tile_pool(name="outp", bufs=1))
        e_pool = phase_b.enter_context(tc.tile_pool(name="epool", bufs=13))
        u_pool = phase_b.enter_context(tc.tile_pool(name="usk", bufs=2))
        r_pool = phase_b.enter_context(tc.tile_pool(name="rsb", bufs=2))
        ln_pool = phase_b.enter_context(tc.tile_pool(name="lnp", bufs=8))
        ln1_pool = phase_b.enter_context(tc.tile_pool(name="lnp1", bufs=1))
        stg_pool = phase_b.enter_context(tc.tile_pool(name="stg", bufs=2))
        OUT_int = outp.tile([128, NQS * 2 * D], F32)
        stat_raw = ln1_pool.tile([128, NQS * 2 * H * 6], F32, tag="sraw")

        def scores_pair(h, kp):
            """One e-pair tile [128, 2048] fp8: exp(scores) of kt=2*kp
            (cols 0:1024) and kt=2*kp+1 (1024:2048)."""
            epr = e_pool.tile([128, 2048], F8, tag="e")
            for sub in range(2):
                kt = 2 * kp + sub
                scp = sc_psum.tile([128, 1024], F32, tag="sc")
                klhs = K_stk[:, h * S + kt * 128 : h * S + kt * 128 + 128]
                nc.tensor.matmul(
                    scp[:, 0:512], klhs, Qv1[:, h * TQ : (h + 1) * TQ],
                    start=True, stop=True,
                )
                nc.tensor.matmul(
                    scp[:, 512:1024], klhs, Qv2[:, h * TQ : (h + 1) * TQ],
                    start=True, stop=True,
                )
                nc.scalar.activation(
                    out=epr[:, sub * 1024 : (sub + 1) * 1024], in_=scp,
                    func=mybir.ActivationFunctionType.Exp,
                    scale=SCALE,
                )
            return epr

        def scores_stage(h):
            return [scores_pair(h, kp) for kp in range(NKT // 2)]

        # V weights on the scalar queue, emitted after wq so they don't
        # steal early DMA bandwidth from the critical xk/xq loads.
        wv = load_w(wv_pool, wv_d, "wv", False, eng=nc.scalar)
        xvT = xtv_pool.tile([128, NCH * S], F8, tag="xvT")

        def load_xv_tile(ts_):
            nc.sync.dma_start(
                out=xvT[:, ts_ * 1024 : (ts_ + 1) * 1024],
                in_=xv_d[:, ts_],
            )

        bv_bc = consts.tile([128, 2 * D], F32)
        gam_bc = consts.tile([128, 2 * D], F32)
        bet_bc = consts.tile([128, 2 * D], F32)
        bcs = [(bv_d, bv_bc)]
        if ln_affine:
            bcs += [(gam_d, gam_bc), (bet_d, bet_bc)]
        for dr, bc in bcs:
            ap0 = dr[:]
            src = bass.AP(tensor=ap0.tensor, offset=0, ap=[[0, 128], [1, 2 * D]])
            nc.gpsimd.dma_start(out=bc, in_=src)

        # first two jobs' scores run while the V input still streams
        ets_live = [None] * H
        for ts_ in range(NKT // 2):
            load_xv_tile(ts_)
        ets_live[0] = scores_stage(0)
        for ts_ in range(NKT // 2, NKT):
            load_xv_tile(ts_)
        ets_live[1] = scores_stage(1)

        # residual (bf16) on the scalar (ACT) queue, emitted after the xv
        # natural tiles so this 1MB load stays out of the startup and
        # scores-phase bandwidth (first use is av_stage(0)). The deferred
        # V-bias is folded in once, so the per-job bias adds disappear
        # from phase B.
        resid_r = consts.tile([128, NQS, D], BF16)
        resid_i = consts.tile([128, NQS, D], BF16)
        for xd, rt, off in ((rq_r, resid_r, 0), (rq_i, resid_i, 1)):
            nc.scalar.dma_start(out=rt, in_=xd[:])
            bvp = bass.AP(
                tensor=bv_bc.tensor, offset=bv_bc.offset + off,
                ap=[bv_bc.ap[0], [2, D]],
            )
            for q4 in range(NQS):
                nc.vector.tensor_add(
                    out=rt[:, q4], in0=rt[:, q4], in1=bvp
                )

        # -- V stage (full S, all heads)
        ps_v = ExitStack()
        vp_psum = ps_v.enter_context(
            tc.tile_pool(name="vp_ps", bufs=2, space="PSUM")
        )
        for ts_ in range(NKT):
            ps = vp_psum.tile([128, 1024], F32, tag="vp")
            for pr in range(NPR):
                lhsT = bass.AP(
                    tensor=xvT.tensor,
                    offset=xvT.offset + ts_ * 1024 + (2 * pr) * 128,
                    ap=[xvT.ap[0], [128, 2], [1, 128]],
                )
                for half in range(2):
                    nc.tensor.matmul(
                        ps[:, half * 512 : (half + 1) * 512],
                        lhsT,
                        w_pair(wv, half * 512, pr, ncol=512),
                        start=(pr == 0),
                        stop=(pr == NPR - 1),
                        perf_mode=DR,
                    )
            # scatter into head-paired layout [Vr_h 64 | Vi_h 64]
            vt = V_pairs[ts_ // 2]
            dst = bass.AP(
                tensor=vt.tensor,
                offset=vt.offset + (ts_ % 2) * 1024,
                ap=[vt.ap[0], [128, H], [DH, 2], [1, DH]],
            )
            nc.vector.tensor_copy(
                out=dst,
                in_=ps.rearrange("p (c h j) -> p h c j", c=2, h=H),
            )
        ps_v.close()  # free V psum banks for p12/raux
        p12_psum = phase_b.enter_context(
            tc.tile_pool(name="p12_ps", bufs=1, space="PSUM")
        )
        r_psum = phase_b.enter_context(
            tc.tile_pool(name="r_ps", bufs=1, space="PSUM")
        )

        def av_chunk(h, eprs, q0, nq4, final=False, sc_h=None):
            """AV + normalize + output for queries [q0*128, (q0+nq4)*128).
            Normal jobs run one full chunk (q0=0, nq4=4); the final job
            runs two half chunks so the first half's LayerNorm + store
            overlaps the second half's matmuls. When sc_h is given, the
            next job's scores matmuls are interleaved at kt-pair
            granularity so the PE has AV/rsum work to chew while exp
            drains each scores psum buffer (otherwise the scores ring
            blocks the PE ~0.6us per pair)."""
            QB = q0 * 128
            NQ = nq4 * 128
            p12 = p12_psum.tile([128, 1024], F32, tag="p12")
            rp = r_psum.tile([1, 1024], F32, tag="raux")
            ones_pair = bass.AP(
                tensor=ones_8.tensor, offset=ones_8.offset,
                ap=[ones_8.ap[0], [16, 2], [1, 1]],
            )
            eprs_new = [] if sc_h is not None else None
            for kp in range(NKT // 2):
                epr = eprs[kp]
                vt = V_pairs[kp]
                vl = bass.AP(
                    tensor=vt.tensor,
                    offset=vt.offset + 128 * h,
                    ap=[vt.ap[0], [1024, 2], [1, 128]],
                )
                for half in range(2):
                    er = bass.AP(
                        tensor=epr.tensor,
                        offset=epr.offset + half * 512 + QB,
                        ap=[epr.ap[0], [1024, 2], [1, NQ]],
                    )
                    nc.tensor.matmul(
                        p12[:, half * 512 + QB : half * 512 + QB + NQ],
                        vl, er,
                        start=(kp == 0), stop=(kp == NKT // 2 - 1),
                        perf_mode=DR,
                    )
                for half in range(2):
                    er = bass.AP(
                        tensor=epr.tensor,
                        offset=epr.offset + half * 512 + QB,
                        ap=[epr.ap[0], [1024, 2], [1, NQ]],
                    )
                    nc.tensor.matmul(
                        rp[0:1, half * 512 + QB : half * 512 + QB + NQ],
                        ones_pair, er,
                        start=(kp == 0), stop=(kp == NKT // 2 - 1),
                        perf_mode=DR,
                    )
            # evacuate P12 + r (P12 on ACT: full-rate PSUM reads, and the
            # DVE is the busier engine in this phase; gpsimd can't touch
            # PSUM at all)
            usk = u_pool.tile([128, 1024], BF16, tag="usk")
            pair_ap = lambda t: bass.AP(
                tensor=t.tensor, offset=t.offset + QB,
                ap=[t.ap[0], [512, 2], [1, NQ]],
            )
            # split the P12 evacuation across ACT and DVE: the ACT
            # queue holds the next job's exp chain and the DVE queue the
            # previous job's combine passes — half on each lets the p12
            # ring resume on the faster drain
            nc.scalar.copy(
                out=usk[:, QB : QB + NQ], in_=p12[:, QB : QB + NQ]
            )
            nc.vector.tensor_copy(
                out=usk[:, 512 + QB : 512 + QB + NQ],
                in_=p12[:, 512 + QB : 512 + QB + NQ],
            )
            rtmp = r_pool.tile([1, 1024], F32R, tag="rtmp")
            # DVE: the last ACT op in the ring-critical chain would sit
            # behind the next job's exp queue; DVE is drained here
            nc.vector.tensor_copy(out=pair_ap(rtmp), in_=pair_ap(rp))
            rsb = r_pool.tile([2, 512], F32R, tag="rsb")
            # sync queue: idle in phase B until the tail stores, while
            # the gpsimd engine queue holds ~2.4us of residual-add
            # backlog per job ahead of this latency-critical copy
            nc.sync.dma_start(
                out=rsb[:, QB : QB + NQ],
                in_=bass.AP(
                    tensor=rtmp.tensor, offset=rtmp.offset + QB,
                    ap=[rtmp.ap[0], [512, 2], [1, NQ]],
                ),
            )
            # r transposes -> rinv per qs: [1/r_r, 1/r_i, -1/r_i]
            utp_r = p12_psum.tile([128, 8], F32R, tag="p12")
            rinv = r_pool.tile([128, 12], F32, tag="rinv")
            for q4 in range(q0, q0 + nq4):
                nc.tensor.transpose(
                    utp_r[:, q4 * 2 : q4 * 2 + 2],
                    rsb[:, q4 * 128 : (q4 + 1) * 128],
                    ident_r,
                )
                nc.vector.reciprocal(
                    out=rinv[:, 3 * q4 : 3 * q4 + 2],
                    in_=utp_r[:, q4 * 2 : q4 * 2 + 2].bitcast(F32),
                )
                nc.vector.tensor_scalar_mul(
                    out=rinv[:, 3 * q4 + 2 : 3 * q4 + 3],
                    in0=rinv[:, 3 * q4 + 1 : 3 * q4 + 2],
                    scalar1=-1.0,
                )
            # transpose P12t (2 blocks of 128 per q4)
            utp = r_psum.tile([128, 1024], BF16, tag="raux")
            for q4 in range(q0, q0 + nq4):
                for half in range(2):
                    col = half * 512 + q4 * 128
                    nc.tensor.transpose(
                        utp[:, col : col + 128],
                        usk[:, col : col + 128],
                        ident_b,
                    )
            def pass1(q4):
                # pass 1: head cols (both comps) = P1t * (1/r_r)
                dst = bass.AP(
                    tensor=OUT_int.tensor,
                    offset=OUT_int.offset + q4 * 2 * D + 2 * DH * h,
                    ap=[OUT_int.ap[0], [1, 2], [2, DH]],
                )
                nc.vector.tensor_scalar_mul(
                    out=dst,
                    in0=utp[:, q4 * 128 : (q4 + 1) * 128]
                    .rearrange("p (b c) -> p b c", b=2),
                    scalar1=rinv[:, 3 * q4 : 3 * q4 + 1],
                )

            def pass2(q4):
                # pass 2: r-cols += EiVi * (-1/r_i); i-cols += EiVr / r_i
                for c, sidx in ((0, 2), (1, 1)):
                    dst = bass.AP(
                        tensor=OUT_int.tensor,
                        offset=OUT_int.offset + q4 * 2 * D + 2 * DH * h + c,
                        ap=[OUT_int.ap[0], [2, DH]],
                    )
                    src_col = 512 + q4 * 128 + (1 - c) * DH
                    nc.vector.scalar_tensor_tensor(
                        out=dst,
                        in0=utp[:, src_col : src_col + DH],
                        scalar=rinv[:, 3 * q4 + sidx : 3 * q4 + sidx + 1],
                        in1=dst,
                        op0=mybir.AluOpType.mult,
                        op1=mybir.AluOpType.add,
                    )

            def resid_stats(q4):
                # residual (bv pre-folded) on gpsimd (SBUF-only engine
                # with slack; pipelines against the DVE bn_stats chain)
                # + LN partial stats (64-dim blocks of this head).
                for c, rt in ((0, resid_r), (1, resid_i)):
                    dst = bass.AP(
                        tensor=OUT_int.tensor,
                        offset=OUT_int.offset + q4 * 2 * D + 2 * DH * h + c,
                        ap=[OUT_int.ap[0], [2, DH]],
                    )
                    nc.gpsimd.tensor_add(
                        out=dst, in0=dst,
                        in1=rt[:, q4, DH * h : DH * (h + 1)],
                    )
                for c in range(2):
                    x = bass.AP(
                        tensor=OUT_int.tensor,
                        offset=OUT_int.offset + q4 * 2 * D + 2 * DH * h + c,
                        ap=[OUT_int.ap[0], [2, DH]],
                    )
                    so = ((q4 * 2 + c) * H + h) * 6
                    nc.vector.bn_stats(
                        out=stat_raw[:, so : so + 6], in_=x
                    )

            if final:
                # fused per-q4 chain so each block's LayerNorm + store
                # starts the moment its stats land
                for q4 in range(q0, q0 + nq4):
                    pass1(q4)
                    pass2(q4)
                    resid_stats(q4)
                    ln_q4(q4)
            else:
                for q4 in range(q0, q0 + nq4):
                    pass1(q4)
                for q4 in range(q0, q0 + nq4):
                    pass2(q4)
                for q4 in range(q0, q0 + nq4):
                    resid_stats(q4)
            return eprs_new

        def ln_q4(q4):
            """LayerNorm + store of one 128-query block. Normalizes into
            a staging tile (no WAR on OUT_int -> stores overlap), real
            part on DVE, imag part on ACT, stores alternate DMA queues."""
            stage = stg_pool.tile([128, 2 * D], BF16, tag="stg")
            for c in range(2):
                so = (q4 * 2 + c) * H * 6
                mv = ln_pool.tile([128, 2], F32, tag="mv")
                nc.vector.bn_aggr(out=mv, in_=stat_raw[:, so : so + H * 6])
                rs = ln_pool.tile([128, 1], F32, tag="rs")
                nc.scalar.activation(
                    out=rs, in_=mv[:, 1:2],
                    func=mybir.ActivationFunctionType.Sqrt,
                    bias=eps_t,
                )
                nc.vector.reciprocal(out=rs, in_=rs)
                x_in = bass.AP(
                    tensor=OUT_int.tensor,
                    offset=OUT_int.offset + q4 * 2 * D + c,
                    ap=[OUT_int.ap[0], [2, D]],
                )
                x_out = bass.AP(
                    tensor=stage.tensor,
                    offset=stage.offset + c,
                    ap=[stage.ap[0], [2, D]],
                )
                if c == 0:
                    nc.vector.tensor_scalar(
                        out=x_out, in0=x_in, scalar1=mv[:, 0:1], scalar2=rs,
                        op0=mybir.AluOpType.subtract,
                        op1=mybir.AluOpType.mult,
                    )
                else:
                    nmr = ln_pool.tile([128, 1], F32, tag="nmr")
                    nc.vector.tensor_scalar(
                        out=nmr, in0=mv[:, 0:1], scalar1=rs, scalar2=-1.0,
                        op0=mybir.AluOpType.mult, op1=mybir.AluOpType.mult,
                    )
                    nc.scalar.activation(
                        out=x_out, in_=x_in,
                        func=mybir.ActivationFunctionType.Identity,
                        bias=nmr, scale=rs,
                    )
            if ln_affine:
                nc.vector.tensor_mul(out=stage, in0=stage, in1=gam_bc)
                nc.vector.tensor_add(out=stage, in0=stage, in1=bet_bc)
            eng = nc.sync if q4 % 2 == 0 else nc.scalar
            eng.dma_start(
                out=out_d[q4 * 128 : (q4 + 1) * 128],
                in_=stage.rearrange("p (d c) -> p d c", c=2),
            )

        for j in range(2, H):
            av_chunk(j - 2, ets_live[j - 2], 0, 4)
            ets_live[j - 2] = None
            ets_live[j] = scores_stage(j)
        av_chunk(H - 2, ets_live[H - 2], 0, 4)
        av_chunk(H - 1, ets_live[H - 1], 0, 3, final=True)
        av_chunk(H - 1, ets_live[H - 1], 3, 1, final=True)
        phase_b.close()
        st_v.close()
    nc.compile()
    return nc


F8NP = ml_dtypes.float8_e4m3fn


def _prep_w_qk(W: np.ndarray) -> np.ndarray:
    """W [D, D, 2] -> [D, 1024] fp8 (patterns 1|2, 8 heads paired)."""
    wr = np.ascontiguousarray(W[:, :, 0].T)  # [d_in, e]
    wi = np.ascontiguousarray(W[:, :, 1].T)

    def paired(a, b):
        out = np.empty((D, 1024), np.float32)
        for h in range(H):
            out[:, 128 * h : 128 * h + DH] = a[:, DH * h : DH * (h + 1)]
            out[:, 128 * h + DH : 128 * h + 128] = b[:, DH * h : DH * (h + 1)]
        return out

    w = paired(wr, wi)
    return np.ascontiguousarray(w).astype(F8NP)


def _prep_w_qk_hm(W: np.ndarray) -> np.ndarray:
    """W [D, D, 2] -> [H, 128, 512] fp8: head-major pattern1 only;
    out[h, p, ds*128 + j] = paired[ds*128 + p, 128h + j]."""
    w = _prep_w_qk(W)  # [D, 1024] fp8
    t = w.reshape(4, 128, 8, 128).transpose(2, 1, 0, 3)
    return np.ascontiguousarray(t.reshape(H, 128, 512))


def _prep_w_v(W: np.ndarray) -> np.ndarray:
    """W [D, D, 2] -> [D, 1024] fp8: lo pattern (Wr^T|Wi^T) only."""
    wr = np.ascontiguousarray(W[:, :, 0].T)
    wi = np.ascontiguousarray(W[:, :, 1].T)
    w = np.concatenate([wr, wi], axis=1)
    return np.ascontiguousarray(w).astype(F8NP)


def _stk(vr, vi):
    """bias vectors -> [128, H]: rows 0:64 vr per head, 64:128 vi."""
    out = np.empty((128, H), np.float32)
    for h in range(H):
        out[0:DH, h] = vr[h * DH : (h + 1) * DH]
        out[DH:128, h] = vi[h * DH : (h + 1) * DH]
    return out


def _inter(a, b):
    out = np.empty((1, 2 * D), np.float32)
    out[0, 0::2] = a
    out[0, 1::2] = b
    return out


def host_inputs(inputs: dict) -> list[dict]:
    q = np.asarray(inputs["q"], np.float32)
    k = np.asarray(inputs["k"], np.float32)
    v = np.asarray(inputs["v"], np.float32)
    Wq, bq = np.asarray(inputs["Wq"], np.float32), np.asarray(inputs["bq"], np.float32)
    Wk, bk = np.asarray(inputs["Wk"], np.float32), np.asarray(inputs["bk"], np.float32)
    Wv, bv = np.asarray(inputs["Wv"], np.float32), np.asarray(inputs["bv"], np.float32)

    shared = {
        "wq": _prep_w_qk(Wq),
        "wk": _prep_w_qk(Wk),
        "wv": _prep_w_v(Wv),
        # sign-folded for the ACT Q evacuation: lower rows hold -bq_i so
        # out = ps*(-1) + (-bq_i) = -(Qi_raw + bq_i)
        "bq_stk": _stk(bq[:, 0], -bq[:, 1]),
        "bk_stk": _stk(bk[:, 0], bk[:, 1]),
        "bv_int": _inter(bv[:, 0] - bv[:, 1], bv[:, 0] + bv[:, 1]),
        "gam_int": _inter(
            np.asarray(inputs["gamma_r"], np.float32),
            np.asarray(inputs["gamma_i"], np.float32),
        ),
        "bet_int": _inter(
            np.asarray(inputs["beta_r"], np.float32),
            np.asarray(inputs["beta_i"], np.float32),
        ),
    }
    def _xT_full(x):
        """x [rows, D, 2] f32 -> [128, NCH, rows] fp8, ch = c*4 + d_blk."""
        a = x.astype(F8NP)            # [rows, D, 2]
        a = a.transpose(1, 2, 0)      # [D, 2, rows]
        a = a.reshape(4, 128, 2, -1)  # [db, p, c, rows]
        a = a.transpose(1, 2, 0, 3)   # [p, c, db, rows]
        return a.reshape(128, NCH, -1)

    kb, vb = {}, {}
    for b_ in range(B):
        kb[b_] = np.ascontiguousarray(
            _xT_full(k[b_]).reshape(128, NCH, 2, 512).transpose(0, 2, 1, 3)
        )
        vb[b_] = np.ascontiguousarray(
            _xT_full(v[b_]).reshape(128, NCH, NKT, 128).transpose(0, 2, 1, 3)
        )

    in_maps = []
    for c in range(NCORES):
        b_, qh = c // 2, c % 2
        qsl = slice(qh * TQ, (qh + 1) * TQ)
        in_maps.append(
            {
                "xqt": np.ascontiguousarray(
                    _xT_full(q[b_, qsl]).reshape(128, NCH * TQ)
                ),
                "xkt": kb[b_], "xvt": vb[b_],
                "rq_r": np.ascontiguousarray(
                    q[b_, qsl, :, 0].reshape(NQS, 128, D).transpose(1, 0, 2)
                ).astype(ml_dtypes.bfloat16),
                "rq_i": np.ascontiguousarray(
                    q[b_, qsl, :, 1].reshape(NQS, 128, D).transpose(1, 0, 2)
                ).astype(ml_dtypes.bfloat16),
                **shared,
            }
        )
    return in_maps


_NC_CACHE = {}
LAST_RESULT = [None]


def kernel(**inputs) -> np.ndarray:
    in_maps = host_inputs(inputs)
    ln_affine = not (
        np.all(np.asarray(inputs["gamma_r"], np.float32) == 1.0)
        and np.all(np.asarray(inputs["gamma_i"], np.float32) == 1.0)
        and np.all(np.asarray(inputs["beta_r"], np.float32) == 0.0)
        and np.all(np.asarray(inputs["beta_i"], np.float32) == 0.0)
    )
    key = ("nc", ln_affine)
    if key not in _NC_CACHE:
        _NC_CACHE[key] = build_nc(ln_affine)
    nc = _NC_CACHE[key]
    res = run_bass_kernel_spmd(nc, in_maps, list(range(NCORES)))
    LAST_RESULT[0] = res
    out = np.empty((B, S, D, 2), np.float32)
    for c in range(NCORES):
        b_, qh = c // 2, c % 2
        out[b_, qh * TQ : (qh + 1) * TQ] = np.asarray(
            res.results[c]["out"], dtype=np.float32
        )
    return out


# revision 64
# speedup vs baseline: 1.0165x; 1.0165x over previous
"""Complex multihead attention Trainium2 kernel (no collectives).

Sharding: core c = (batch b=c//2, query-half qh=c%2). Each core computes
K/V projections for the full sequence (all 8 heads), Q projection for
its 512 queries, attention for all heads over its queries, and a fully
local residual + LayerNorm over the full 512 dims. No cross-core
communication (collectives in this runtime cost 30-50us of unhidable
tail latency, more than the duplicated K/V projection work).

Compute dtype: fp8e4 (e4m3) operands on the PE with DoubleRow perf mode
(two 128-deep contraction tiles per matmul instruction) for every
contraction >= 256: the Q/K/V projections (contraction 1024 = 4 pairs),
the AV application and the softmax-denominator rsum (contraction over
S=1024 keys = 4 kt pairs). Scores (contraction 128 = r/i x dh) stay
plain-mode fp8. f32 PSUM accumulation throughout; the residual path
stays bf16 and the LayerNorm f32, so fp8 noise in the attention path is
diluted ~20x by the residual before the final normalization.

Inputs arrive as interleaved fp8 [rows, D, 2]; PE-transposes (1 cyc/row)
build the stacked-transposed layouts (XBAR dma_start_transpose moves
data in 256B packets here and is ~4x slower than plain DMA). Weights are
loaded compact (pattern1 only) and pattern2 = (-Wi^T | Wr^T) is derived
on the otherwise-idle DVE. Qv2 is derived from Qv1 by a partition-swap
DMA + negate instead of a second projection matmul.

Natural-tile loads alternate the sync/scalar hwdge queues.
DMA queue discipline (queues are in-order; a dependency-stalled DMA at
the head blocks everything behind it): sync carries the natural-tile x
loads + output stores, gpsimd (software DGE, async transfers) carries
weights/broadcasts/residual and the tiny per-job rsb copies, scalar
carries only the Qv2 partition-swap.

Phase B runs 8 jobs (one per head, 512 queries), software-pipelined
with depth 2: scores of jobs j+1/j+2 are issued before AV of job j so
the Scalar-engine exp chain overlaps the PE's AV/rsum work (the PE
queue is in-order; without this the PE head-of-line blocks on exp).
The first two jobs' scores are issued before the V projection so exp
hides behind V matmuls. Residual add + deferred V-bias + LayerNorm
partial stats (bn_stats on 64-dim head blocks) run per job, overlapped;
the tail only aggregates (bn_aggr), normalizes and stores. The LN
affine (gamma/beta) ops are compiled out when gamma==1 and beta==0.

On-chip layouts (per core, S=1024 keys, TQ=512 queries):
  xkT/xvT [128, 8ch*S] fp8, xqT [128, 8ch*TQ]: ch<4 -> Xr^T d-block,
        ch>=4 -> Xi^T (stacked transposed inputs).
  wq/wk [128, 4ds*2048] fp8: per d_sub, pattern1 (Wr^T|Wi^T)
        head-paired cols 0:1024, pattern2 (-Wi^T|Wr^T) 1024:2048.
  wv    [128, 4ds*2048] fp8: per d_sub, lo (Wr^T|Wi^T) plain 0:1024,
        hi (-Wi^T|Wr^T) 1024:2048.
  K_stk [128, 8h*S] fp8: rows 0:64 Kr^T, 64:128 Ki^T per head (+bias).
  Qv1   [128, 8h*TQ] fp8: [Qr^T; -Qi^T] per head (bias folded).
  Qv2   [128, 8h*TQ] fp8: [Qi^T; Qr^T].
  V_all [128, 8kt*1024] fp8: per k-tile, head-paired [Vr_h 64|Vi_h 64].
  scores psum [128 k, 1024] f32 per (job, kt): 0:512 Sr^T, 512: Si^T
  E = exp(0.125*S^T) sbuf fp8 [128, 2048] per kt PAIR (DoubleRow rhs)
  P12 psum [128, 1024] f32 = sum_kp V^T [Er|Ei] -> [ErVr;ErVi|EiVr;EiVi]
  rp psum [1, 1024] f32 = ones^T [Er | Ei] (softmax denominators)
  OUT_int [128, 4qs*1024] f32: (d,c)-interleaved full-D rows=q
  stat_raw [128, 4qs*2c*8h*6] f32: per-job bn_stats partials.
"""

import math
from contextlib import ExitStack

import numpy as np
import ml_dtypes

import concourse.bass as bass
import concourse.tile as tile
from concourse import bacc, mybir
from concourse.bass_utils import run_bass_kernel_spmd
from concourse.masks import make_identity

B, S, D, H = 4, 1024, 512, 8
DH = D // H  # 64
TQ = S // 2  # 512 queries per core
NCORES = 8
EPS = 1e-5
F32 = mybir.dt.float32
F32R = mybir.dt.float32r
BF16 = mybir.dt.bfloat16
F8 = mybir.dt.float8e4
DR = mybir.MatmulPerfMode.DoubleRow

NKT = S // 128  # 8 key tiles
NQS = TQ // 128  # 4 query subtiles
NCH = 8  # contraction chunks (2*D/128)
NPR = 4  # DoubleRow contraction pairs
SCALE = 1.0 / math.sqrt(DH)


def build_nc(ln_affine: bool = True) -> bass.Bass:
    nc = bacc.Bacc(None, target_bir_lowering=False, debug=False)

    # x inputs arrive pre-transposed from the host into the stacked
    # layout (ch = c*4 + d_block, partition = d-within-block): no on-chip
    # transposes, and the layouts are sliced so each compute stage's DMA
    # lands just ahead of its matmuls (xk by 512-query half, xv by
    # 128-key tile).
    xq_d = nc.declare_dram_parameter("xqt", [128, NCH * TQ], F8, isOutput=False)
    xk_d = nc.declare_dram_parameter("xkt", [128, 2, NCH, 512], F8, isOutput=False)
    xv_d = nc.declare_dram_parameter("xvt", [128, NKT, NCH, 128], F8, isOutput=False)
    rq_r = nc.declare_dram_parameter("rq_r", [128, NQS, D], BF16, isOutput=False)
    rq_i = nc.declare_dram_parameter("rq_i", [128, NQS, D], BF16, isOutput=False)
    wq_d = nc.declare_dram_parameter("wq", [D, 1024], F8, isOutput=False)
    wk_d = nc.declare_dram_parameter("wk", [D, 1024], F8, isOutput=False)
    wv_d = nc.declare_dram_parameter("wv", [D, 1024], F8, isOutput=False)
    bq_d = nc.declare_dram_parameter("bq_stk", [128, H], F32, isOutput=False)
    bk_d = nc.declare_dram_parameter("bk_stk", [128, H], F32, isOutput=False)
    bv_d = nc.declare_dram_parameter("bv_int", [1, 2 * D], F32, isOutput=False)
    gam_d = nc.declare_dram_parameter("gam_int", [1, 2 * D], F32, isOutput=False)
    bet_d = nc.declare_dram_parameter("bet_int", [1, 2 * D], F32, isOutput=False)
    # output stored bf16 (host upconverts): halves the 2MB store and
    # the tail's last-store latency; adds <=0.4% quantization on the
    # normalized output, well inside the error budget.
    out_d = nc.declare_dram_parameter("out", [TQ, D, 2], BF16, isOutput=True)

    with tile.TileContext(nc) as tc, ExitStack() as ctx:
        consts = ctx.enter_context(tc.tile_pool(name="consts", bufs=1))
        attn_in = ctx.enter_context(tc.tile_pool(name="attn_in", bufs=1))

        ident_f = consts.tile([128, 128], F32)
        ident_b = consts.tile([128, 128], BF16)
        ident_r = consts.tile([2, 2], F32R)
        # dual-fp8 ldweights requires the pair dim's step % 16 == 0:
        # keep the two ones columns 16 apart.
        ones_f = consts.tile([128, 32], F32)
        nc.vector.memset(ones_f, 1.0)
        ones_8 = consts.tile([128, 32], F8)
        nc.vector.tensor_copy(out=ones_8, in_=ones_f)
        eps_t = consts.tile([128, 1], F32)
        nc.vector.memset(eps_t, EPS)
        # per-partition sign vector for the Q evacuation on ACT:
        # rows 0:64 -> +1 (real part), rows 64:128 -> -1 (negated imag)
        sign_t = consts.tile([128, 1], F32)
        nc.vector.memset(sign_t, 1.0)
        nc.vector.memset(sign_t[DH:128, :], -1.0)

        # ---- DMA queue discipline ----
        # sync (hwdge): xk/xq/xv stacked loads + even stores; scalar
        # (hwdge): weights, Qv2 swap, residual, odd stores; gpsimd
        # (swdge): broadcasts + per-job rsb copies.
        st_v = ExitStack()
        st_q = ExitStack()
        st_k = ExitStack()
        xtv_pool = st_v.enter_context(tc.tile_pool(name="xtv", bufs=1))
        wv_pool = st_v.enter_context(tc.tile_pool(name="wvp", bufs=1))
        xtq_pool = st_q.enter_context(tc.tile_pool(name="xtq", bufs=1))
        wq_pool = st_q.enter_context(tc.tile_pool(name="wqp", bufs=1))
        xtk_pool = st_k.enter_context(tc.tile_pool(name="xtk", bufs=1))
        wk_pool = st_k.enter_context(tc.tile_pool(name="wkp", bufs=1))

        # PSUM pool lifetimes are LIFO per space; 16KB/partition budget:
        #   K/Q phase:   sc 8K + proj 4K          = 12K
        #   V stage:     sc 8K + vp 8K            = 16K
        #   AV phase:    sc 8K + p12 4K + r 4K    = 16K
        sc_psum = ctx.enter_context(
            tc.tile_pool(name="sc_ps", bufs=2, space="PSUM")
        )
        ps_p = ExitStack()
        proj_psum = ps_p.enter_context(
            tc.tile_pool(name="proj_ps", bufs=2, space="PSUM")
        )

        def load_w(w_pool, w_dram, tag, headpaired, eng=None):
            """One 512KB DMA (pattern1, ds-strided in SBUF); derive
            pattern2 = (-Wi | Wr) on DVE."""
            w = w_pool.tile([128, 4 * 2048], F8, tag=tag)
            ap0 = w_dram[:]
            (eng or nc.gpsimd).dma_start(
                out=bass.AP(
                    tensor=w.tensor, offset=w.offset,
                    ap=[w.ap[0], [2048, 4], [1, 1024]],
                ),
                in_=bass.AP(
                    tensor=ap0.tensor, offset=0,
                    ap=[[1024, 128], [128 * 1024, 4], [1, 1024]],
                ),
            )
            blk = 64 if headpaired else 512
            nb = 1024 // (2 * blk)
            def ap_of(col0):
                return bass.AP(
                    tensor=w.tensor,
                    offset=w.offset + col0,
                    ap=[w.ap[0], [2048, 4], [2 * blk, nb], [1, blk]],
                )
            nc.vector.tensor_scalar_mul(
                out=ap_of(1024), in0=ap_of(blk), scalar1=-1.0
            )
            nc.vector.tensor_copy(out=ap_of(1024 + blk), in_=ap_of(0))
            return w

        bq_stk = consts.tile([128, H], F32)
        nc.gpsimd.dma_start(out=bq_stk, in_=bq_d[:])
        bk_stk = consts.tile([128, H], F32)
        nc.gpsimd.dma_start(out=bk_stk, in_=bk_d[:])

        # attention-phase operand tensors
        K_stk = attn_in.tile([128, H * S], F8)
        V_pairs = [
            attn_in.tile([128, 2 * 1024], F8, name=f"vpair{i}",
                         tag=f"vpair{i}")
            for i in range(NKT // 2)
        ]
        Qv1 = attn_in.tile([128, H * TQ], F8)
        Qv2 = attn_in.tile([128, H * TQ], F8)

        def w_pair(w, col0, pr, ncol=128):
            """DoubleRow operand: chunk pair (2*pr, 2*pr+1) = adjacent
            d_subs of pattern pr//2, columns col0:col0+ncol."""
            base = (pr % 2) * 2 * 2048 + (pr // 2) * 1024 + col0
            return bass.AP(
                tensor=w.tensor,
                offset=w.offset + base,
                ap=[w.ap[0], [2048, 2], [1, ncol]],
            )

        # ---------------- phase A: projections --------------------------
        # -- K stage, split by 512-key halves: each half's matmuls start
        # as soon as its 512KB DMA slice lands. wk rides the scalar hwdge
        # queue (it gates the first matmul; the swdge gpsimd queue is
        # ~4x slower to first byte).
        wk = load_w(wk_pool, wk_d, "wk", True, eng=nc.scalar)
        # SBUF layout = host layout [tch][ch][512]: the DMA stays fully
        # contiguous (scattered-destination DMAs run at <1/4 bandwidth);
        # the DoubleRow pair AP just uses stride 512 instead of S.
        xkT = xtk_pool.tile([128, 2 * NCH * 512], F8, tag="xkT")

        def load_xk_half(tch):
            if tch == 0:
                # split the first (critical) half across two queues
                nc.sync.dma_start(
                    out=xkT[:, 0:2048], in_=xk_d[:, 0, 0:4],
                )
                nc.gpsimd.dma_start(
                    out=xkT[:, 2048:4096], in_=xk_d[:, 0, 4:8],
                )
            else:
                nc.sync.dma_start(
                    out=xkT[:, tch * 4096 : (tch + 1) * 4096],
                    in_=xk_d[:, tch],
                )

        def xk_pair(pr, tch):
            return bass.AP(
                tensor=xkT.tensor,
                offset=xkT.offset + tch * 4096 + (2 * pr) * 512,
                ap=[xkT.ap[0], [512, 2], [1, 512]],
            )

        def k_half(tch):
            for h in range(H):
                ps = proj_psum.tile([128, 512], F32, tag="proj")
                for pr in range(NPR):
                    nc.tensor.matmul(
                        ps,
                        w_pair(wk, 128 * h, pr),
                        xk_pair(pr, tch),
                        start=(pr == 0),
                        stop=(pr == NPR - 1),
                        perf_mode=DR,
                    )
                nc.scalar.activation(
                    out=K_stk[:, h * S + tch * 512 : h * S + (tch + 1) * 512],
                    in_=ps,
                    func=mybir.ActivationFunctionType.Identity,
                    bias=bk_stk[:, h : h + 1],
                )

        load_xk_half(0)
        k_half(0)
        load_xk_half(1)
        # identity tables (for the P12t/r transposes much later) are
        # built here so their DMAs queue behind the critical xk loads
        make_identity(nc, ident_f)
        nc.vector.tensor_copy(out=ident_b, in_=ident_f)
        nc.vector.tensor_copy(out=ident_r, in_=ident_f[0:2, 0:2])
        k_half(1)
        st_k.close()  # free xkT + wk SBUF

        # -- Q stage (8 heads, TQ queries; Qv1 only; Qv2 derived).
        # Evacuation fused on ACT: out = ps*sign + bias with per-partition
        # sign (+1 top / -1 bottom) and sign-folded bias from the host.
        wq = load_w(wq_pool, wq_d, "wq", True, eng=nc.scalar)
        xqT = xtq_pool.tile([128, NCH * TQ], F8, tag="xqT")
        nc.sync.dma_start(out=xqT, in_=xq_d[:])
        for h in range(H):
            ps = proj_psum.tile([128, TQ], F32, tag="proj")
            for pr in range(NPR):
                nc.tensor.matmul(
                    ps,
                    w_pair(wq, 128 * h, pr),
                    bass.AP(
                        tensor=xqT.tensor,
                        offset=xqT.offset + (2 * pr) * 512,
                        ap=[xqT.ap[0], [512, 2], [1, 512]],
                    ),
                    start=(pr == 0),
                    stop=(pr == NPR - 1),
                    perf_mode=DR,
                )
            nc.scalar.activation(
                out=Qv1[:, h * TQ : (h + 1) * TQ],
                in_=ps,
                func=mybir.ActivationFunctionType.Identity,
                bias=bq_stk[:, h : h + 1],
                scale=sign_t,
            )
        # Qv2 = [Qi; Qr] = [-Qv1_bot ; Qv1_top]
        nc.scalar.dma_start(out=Qv2[DH:128, :], in_=Qv1[0:DH, :])
        nc.scalar.dma_start(out=Qv2[0:DH, :], in_=Qv1[DH:128, :])
        nc.vector.tensor_scalar_mul(
            out=Qv2[0:DH, :], in0=Qv2[0:DH, :], scalar1=-1.0
        )
        ps_p.close()  # free projection psum banks
        st_q.close()  # free xqT + wq SBUF

        # ---------------- phase B pools (scores pool opens early) --------
        phase_b = ExitStack()
        outp = phase_b.enter_context(tc.tile_pool(name="outp", bufs=1))
        e_pool = phase_b.enter_context(tc.tile_pool(name="epool", bufs=13))
        u_pool = phase_b.enter_context(tc.tile_pool(name="usk", bufs=2))
        r_pool = phase_b.enter_context(tc.tile_pool(name="rsb", bufs=2))
        ln_pool = phase_b.enter_context(tc.tile_pool(name="lnp", bufs=8))
        ln1_pool = phase_b.enter_context(tc.tile_pool(name="lnp1", bufs=1))
        stg_pool = phase_b.enter_context(tc.tile_pool(name="stg", bufs=2))
        OUT_int = outp.tile([128, NQS * 2 * D], F32)
        stat_raw = ln1_pool.tile([128, NQS * 2 * H * 6], F32, tag="sraw")

        def scores_pair(h, kp):
            """One e-pair tile [128, 2048] fp8: exp(scores) of kt=2*kp
            (cols 0:1024) and kt=2*kp+1 (1024:2048)."""
            epr = e_pool.tile([128, 2048], F8, tag="e")
            for sub in range(2):
                kt = 2 * kp + sub
                scp = sc_psum.tile([128, 1024], F32, tag="sc")
                klhs = K_stk[:, h * S + kt * 128 : h * S + kt * 128 + 128]
                nc.tensor.matmul(
                    scp[:, 0:512], klhs, Qv1[:, h * TQ : (h + 1) * TQ],
                    start=True, stop=True,
                )
                nc.tensor.matmul(
                    scp[:, 512:1024], klhs, Qv2[:, h * TQ : (h + 1) * TQ],
                    start=True, stop=True,
                )
                nc.scalar.activation(
                    out=epr[:, sub * 1024 : (sub + 1) * 1024], in_=scp,
                    func=mybir.ActivationFunctionType.Exp,
                    scale=SCALE,
                )
            return epr

        def scores_stage(h):
            return [scores_pair(h, kp) for kp in range(NKT // 2)]

        # V weights on the scalar queue, emitted after wq so they don't
        # steal early DMA bandwidth from the critical xk/xq loads.
        wv = load_w(wv_pool, wv_d, "wv", False, eng=nc.scalar)
        xvT = xtv_pool.tile([128, NCH * S], F8, tag="xvT")

        def load_xv_tile(ts_):
            nc.sync.dma_start(
                out=xvT[:, ts_ * 1024 : (ts_ + 1) * 1024],
                in_=xv_d[:, ts_],
            )

        bv_bc = consts.tile([128, 2 * D], F32)
        gam_bc = consts.tile([128, 2 * D], F32)
        bet_bc = consts.tile([128, 2 * D], F32)
        bcs = [(bv_d, bv_bc)]
        if ln_affine:
            bcs += [(gam_d, gam_bc), (bet_d, bet_bc)]
        for dr, bc in bcs:
            ap0 = dr[:]
            src = bass.AP(tensor=ap0.tensor, offset=0, ap=[[0, 128], [1, 2 * D]])
            nc.gpsimd.dma_start(out=bc, in_=src)

        # first two jobs' scores run while the V input still streams
        ets_live = [None] * H
        for ts_ in range(NKT // 2):
            load_xv_tile(ts_)
        ets_live[0] = scores_stage(0)
        for ts_ in range(NKT // 2, NKT):
            load_xv_tile(ts_)
        ets_live[1] = scores_stage(1)

        # residual (bf16) on the scalar (ACT) queue, emitted after the xv
        # natural tiles so this 1MB load stays out of the startup and
        # scores-phase bandwidth (first use is av_stage(0)). The deferred
        # V-bias is folded in once, so the per-job bias adds disappear
        # from phase B.
        resid_r = consts.tile([128, NQS, D], BF16)
        resid_i = consts.tile([128, NQS, D], BF16)
        for xd, rt, off in ((rq_r, resid_r, 0), (rq_i, resid_i, 1)):
            nc.scalar.dma_start(out=rt, in_=xd[:])
            bvp = bass.AP(
                tensor=bv_bc.tensor, offset=bv_bc.offset + off,
                ap=[bv_bc.ap[0], [2, D]],
            )
            for q4 in range(NQS):
                nc.vector.tensor_add(
                    out=rt[:, q4], in0=rt[:, q4], in1=bvp
                )

        # -- V stage (full S, all heads)
        ps_v = ExitStack()
        vp_psum = ps_v.enter_context(
            tc.tile_pool(name="vp_ps", bufs=2, space="PSUM")
        )
        for ts_ in range(NKT):
            ps = vp_psum.tile([128, 1024], F32, tag="vp")
            for pr in range(NPR):
                lhsT = bass.AP(
                    tensor=xvT.tensor,
                    offset=xvT.offset + ts_ * 1024 + (2 * pr) * 128,
                    ap=[xvT.ap[0], [128, 2], [1, 128]],
                )
                for half in range(2):
                    nc.tensor.matmul(
                        ps[:, half * 512 : (half + 1) * 512],
                        lhsT,
                        w_pair(wv, half * 512, pr, ncol=512),
                        start=(pr == 0),
                        stop=(pr == NPR - 1),
                        perf_mode=DR,
                    )
            # scatter into head-paired layout [Vr_h 64 | Vi_h 64]
            vt = V_pairs[ts_ // 2]
            dst = bass.AP(
                tensor=vt.tensor,
                offset=vt.offset + (ts_ % 2) * 1024,
                ap=[vt.ap[0], [128, H], [DH, 2], [1, DH]],
            )
            nc.vector.tensor_copy(
                out=dst,
                in_=ps.rearrange("p (c h j) -> p h c j", c=2, h=H),
            )
        ps_v.close()  # free V psum banks for p12/raux
        p12_psum = phase_b.enter_context(
            tc.tile_pool(name="p12_ps", bufs=1, space="PSUM")
        )
        r_psum = phase_b.enter_context(
            tc.tile_pool(name="r_ps", bufs=1, space="PSUM")
        )

        def av_chunk(h, eprs, q0, nq4, final=False, sc_h=None):
            """AV + normalize + output for queries [q0*128, (q0+nq4)*128).
            Normal jobs run one full chunk (q0=0, nq4=4); the final job
            runs two half chunks so the first half's LayerNorm + store
            overlaps the second half's matmuls. When sc_h is given, the
            next job's scores matmuls are interleaved at kt-pair
            granularity so the PE has AV/rsum work to chew while exp
            drains each scores psum buffer (otherwise the scores ring
            blocks the PE ~0.6us per pair)."""
            QB = q0 * 128
            NQ = nq4 * 128
            p12 = p12_psum.tile([128, 1024], F32, tag="p12")
            rp = r_psum.tile([1, 1024], F32, tag="raux")
            ones_pair = bass.AP(
                tensor=ones_8.tensor, offset=ones_8.offset,
                ap=[ones_8.ap[0], [16, 2], [1, 1]],
            )
            eprs_new = [] if sc_h is not None else None
            for kp in range(NKT // 2):
                epr = eprs[kp]
                vt = V_pairs[kp]
                vl = bass.AP(
                    tensor=vt.tensor,
                    offset=vt.offset + 128 * h,
                    ap=[vt.ap[0], [1024, 2], [1, 128]],
                )
                for half in range(2):
                    er = bass.AP(
                        tensor=epr.tensor,
                        offset=epr.offset + half * 512 + QB,
                        ap=[epr.ap[0], [1024, 2], [1, NQ]],
                    )
                    nc.tensor.matmul(
                        p12[:, half * 512 + QB : half * 512 + QB + NQ],
                        vl, er,
                        start=(kp == 0), stop=(kp == NKT // 2 - 1),
                        perf_mode=DR,
                    )
                for half in range(2):
                    er = bass.AP(
                        tensor=epr.tensor,
                        offset=epr.offset + half * 512 + QB,
                        ap=[epr.ap[0], [1024, 2], [1, NQ]],
                    )
                    nc.tensor.matmul(
                        rp[0:1, half * 512 + QB : half * 512 + QB + NQ],
                        ones_pair, er,
                        start=(kp == 0), stop=(kp == NKT // 2 - 1),
                        perf_mode=DR,
                    )
            # evacuate P12 + r (P12 on ACT: full-rate PSUM reads, and the
            # DVE is the busier engine in this phase; gpsimd can't touch
            # PSUM at all)
            usk = u_pool.tile([128, 1024], BF16, tag="usk")
            pair_ap = lambda t: bass.AP(
                tensor=t.tensor, offset=t.offset + QB,
                ap=[t.ap[0], [512, 2], [1, NQ]],
            )
            # both P12 evacuation halves on DVE (drained queue; the ACT
            # queue holds the next job's exp chain and the p12 ring waits
            # BOTH halves). P1 half first so its transposes start early.
            nc.vector.tensor_copy(
                out=usk[:, QB : QB + NQ], in_=p12[:, QB : QB + NQ]
            )
            nc.vector.tensor_copy(
                out=usk[:, 512 + QB : 512 + QB + NQ],
                in_=p12[:, 512 + QB : 512 + QB + NQ],
            )
            rtmp = r_pool.tile([1, 1024], F32R, tag="rtmp")
            # DVE: the last ACT op in the ring-critical chain would sit
            # behind the next job's exp queue; DVE is drained here
            nc.vector.tensor_copy(out=pair_ap(rtmp), in_=pair_ap(rp))
            rsb = r_pool.tile([2, 512], F32R, tag="rsb")
            # sync queue: idle in phase B until the tail stores, while
            # the gpsimd engine queue holds ~2.4us of residual-add
            # backlog per job ahead of this latency-critical copy
            nc.sync.dma_start(
                out=rsb[:, QB : QB + NQ],
                in_=bass.AP(
                    tensor=rtmp.tensor, offset=rtmp.offset + QB,
                    ap=[rtmp.ap[0], [512, 2], [1, NQ]],
                ),
            )
            # r transposes -> rinv per qs: [1/r_r, 1/r_i, -1/r_i]
            utp_r = p12_psum.tile([128, 8], F32R, tag="p12")
            rinv = r_pool.tile([128, 12], F32, tag="rinv")
            for q4 in range(q0, q0 + nq4):
                nc.tensor.transpose(
                    utp_r[:, q4 * 2 : q4 * 2 + 2],
                    rsb[:, q4 * 128 : (q4 + 1) * 128],
                    ident_r,
                )
                nc.vector.reciprocal(
                    out=rinv[:, 3 * q4 : 3 * q4 + 2],
                    in_=utp_r[:, q4 * 2 : q4 * 2 + 2].bitcast(F32),
                )
                nc.vector.tensor_scalar_mul(
                    out=rinv[:, 3 * q4 + 2 : 3 * q4 + 3],
                    in0=rinv[:, 3 * q4 + 1 : 3 * q4 + 2],
                    scalar1=-1.0,
                )
            # transpose P12t (2 blocks of 128 per q4)
            utp = r_psum.tile([128, 1024], BF16, tag="raux")
            for q4 in range(q0, q0 + nq4):
                for half in range(2):
                    col = half * 512 + q4 * 128
                    nc.tensor.transpose(
                        utp[:, col : col + 128],
                        usk[:, col : col + 128],
                        ident_b,
                    )
            def pass1(q4):
                # pass 1: head cols (both comps) = P1t * (1/r_r)
                dst = bass.AP(
                    tensor=OUT_int.tensor,
                    offset=OUT_int.offset + q4 * 2 * D + 2 * DH * h,
                    ap=[OUT_int.ap[0], [1, 2], [2, DH]],
                )
                nc.vector.tensor_scalar_mul(
                    out=dst,
                    in0=utp[:, q4 * 128 : (q4 + 1) * 128]
                    .rearrange("p (b c) -> p b c", b=2),
                    scalar1=rinv[:, 3 * q4 : 3 * q4 + 1],
                )

            def pass2(q4):
                # pass 2: r-cols += EiVi * (-1/r_i); i-cols += EiVr / r_i
                for c, sidx in ((0, 2), (1, 1)):
                    dst = bass.AP(
                        tensor=OUT_int.tensor,
                        offset=OUT_int.offset + q4 * 2 * D + 2 * DH * h + c,
                        ap=[OUT_int.ap[0], [2, DH]],
                    )
                    src_col = 512 + q4 * 128 + (1 - c) * DH
                    nc.vector.scalar_tensor_tensor(
                        out=dst,
                        in0=utp[:, src_col : src_col + DH],
                        scalar=rinv[:, 3 * q4 + sidx : 3 * q4 + sidx + 1],
                        in1=dst,
                        op0=mybir.AluOpType.mult,
                        op1=mybir.AluOpType.add,
                    )

            def resid_stats(q4):
                # residual (bv pre-folded) on gpsimd (SBUF-only engine
                # with slack; pipelines against the DVE bn_stats chain)
                # + LN partial stats (64-dim blocks of this head).
                for c, rt in ((0, resid_r), (1, resid_i)):
                    dst = bass.AP(
                        tensor=OUT_int.tensor,
                        offset=OUT_int.offset + q4 * 2 * D + 2 * DH * h + c,
                        ap=[OUT_int.ap[0], [2, DH]],
                    )
                    nc.gpsimd.tensor_add(
                        out=dst, in0=dst,
                        in1=rt[:, q4, DH * h : DH * (h + 1)],
                    )
                for c in range(2):
                    x = bass.AP(
                        tensor=OUT_int.tensor,
                        offset=OUT_int.offset + q4 * 2 * D + 2 * DH * h + c,
                        ap=[OUT_int.ap[0], [2, DH]],
                    )
                    so = ((q4 * 2 + c) * H + h) * 6
                    nc.vector.bn_stats(
                        out=stat_raw[:, so : so + 6], in_=x
                    )

            if final:
                # fused per-q4 chain so each block's LayerNorm + store
                # starts the moment its stats land
                for q4 in range(q0, q0 + nq4):
                    pass1(q4)
                    pass2(q4)
                    resid_stats(q4)
                    ln_q4(q4)
            else:
                for q4 in range(q0, q0 + nq4):
                    pass1(q4)
                for q4 in range(q0, q0 + nq4):
                    pass2(q4)
                for q4 in range(q0, q0 + nq4):
                    resid_stats(q4)
            return eprs_new

        def ln_q4(q4):
            """LayerNorm + store of one 128-query block. Normalizes into
            a staging tile (no WAR on OUT_int -> stores overlap), real
            part on DVE, imag part on ACT, stores alternate DMA queues."""
            stage = stg_pool.tile([128, 2 * D], BF16, tag="stg")
            for c in range(2):
                so = (q4 * 2 + c) * H * 6
                mv = ln_pool.tile([128, 2], F32, tag="mv")
                nc.vector.bn_aggr(out=mv, in_=stat_raw[:, so : so + H * 6])
                rs = ln_pool.tile([128, 1], F32, tag="rs")
                nc.scalar.activation(
                    out=rs, in_=mv[:, 1:2],
                    func=mybir.ActivationFunctionType.Sqrt,
                    bias=eps_t,
                )
                nc.vector.reciprocal(out=rs, in_=rs)
                x_in = bass.AP(
                    tensor=OUT_int.tensor,
                    offset=OUT_int.offset + q4 * 2 * D + c,
                    ap=[OUT_int.ap[0], [2, D]],
                )
                x_out = bass.AP(
                    tensor=stage.tensor,
                    offset=stage.offset + c,
                    ap=[stage.ap[0], [2, D]],
                )
                if c == 0:
                    nc.vector.tensor_scalar(
                        out=x_out, in0=x_in, scalar1=mv[:, 0:1], scalar2=rs,
                        op0=mybir.AluOpType.subtract,
                        op1=mybir.AluOpType.mult,
                    )
                else:
                    nmr = ln_pool.tile([128, 1], F32, tag="nmr")
                    nc.vector.tensor_scalar(
                        out=nmr, in0=mv[:, 0:1], scalar1=rs, scalar2=-1.0,
                        op0=mybir.AluOpType.mult, op1=mybir.AluOpType.mult,
                    )
                    nc.scalar.activation(
                        out=x_out, in_=x_in,
                        func=mybir.ActivationFunctionType.Identity,
                        bias=nmr, scale=rs,
                    )
            if ln_affine:
                nc.vector.tensor_mul(out=stage, in0=stage, in1=gam_bc)
                nc.vector.tensor_add(out=stage, in0=stage, in1=bet_bc)
            eng = nc.sync if q4 % 2 == 0 else nc.scalar
            eng.dma_start(
                out=out_d[q4 * 128 : (q4 + 1) * 128],
                in_=stage.rearrange("p (d c) -> p d c", c=2),
            )

        for j in range(2, H):
            av_chunk(j - 2, ets_live[j - 2], 0, 4)
            ets_live[j - 2] = None
            ets_live[j] = scores_stage(j)
        av_chunk(H - 2, ets_live[H - 2], 0, 4)
        av_chunk(H - 1, ets_live[H - 1], 0, 3, final=True)
        av_chunk(H - 1, ets_live[H - 1], 3, 1, final=True)
        phase_b.close()
        st_v.close()
    nc.compile()
    return nc


F8NP = ml_dtypes.float8_e4m3fn


def _prep_w_qk(W: np.ndarray) -> np.ndarray:
    """W [D, D, 2] -> [D, 1024] fp8 (patterns 1|2, 8 heads paired)."""
    wr = np.ascontiguousarray(W[:, :, 0].T)  # [d_in, e]
    wi = np.ascontiguousarray(W[:, :, 1].T)

    def paired(a, b):
        out = np.empty((D, 1024), np.float32)
        for h in range(H):
            out[:, 128 * h : 128 * h + DH] = a[:, DH * h : DH * (h + 1)]
            out[:, 128 * h + DH : 128 * h + 128] = b[:, DH * h : DH * (h + 1)]
        return out

    w = paired(wr, wi)
    return np.ascontiguousarray(w).astype(F8NP)


def _prep_w_qk_hm(W: np.ndarray) -> np.ndarray:
    """W [D, D, 2] -> [H, 128, 512] fp8: head-major pattern1 only;
    out[h, p, ds*128 + j] = paired[ds*128 + p, 128h + j]."""
    w = _prep_w_qk(W)  # [D, 1024] fp8
    t = w.reshape(4, 128, 8, 128).transpose(2, 1, 0, 3)
    return np.ascontiguousarray(t.reshape(H, 128, 512))


def _prep_w_v(W: np.ndarray) -> np.ndarray:
    """W [D, D, 2] -> [D, 1024] fp8: lo pattern (Wr^T|Wi^T) only."""
    wr = np.ascontiguousarray(W[:, :, 0].T)
    wi = np.ascontiguousarray(W[:, :, 1].T)
    w = np.concatenate([wr, wi], axis=1)
    return np.ascontiguousarray(w).astype(F8NP)


def _stk(vr, vi):
    """bias vectors -> [128, H]: rows 0:64 vr per head, 64:128 vi."""
    out = np.empty((128, H), np.float32)
    for h in range(H):
        out[0:DH, h] = vr[h * DH : (h + 1) * DH]
        out[DH:128, h] = vi[h * DH : (h + 1) * DH]
    return out


def _inter(a, b):
    out = np.empty((1, 2 * D), np.float32)
    out[0, 0::2] = a
    out[0, 1::2] = b
    return out


def host_inputs(inputs: dict) -> list[dict]:
    q = np.asarray(inputs["q"], np.float32)
    k = np.asarray(inputs["k"], np.float32)
    v = np.asarray(inputs["v"], np.float32)
    Wq, bq = np.asarray(inputs["Wq"], np.float32), np.asarray(inputs["bq"], np.float32)
    Wk, bk = np.asarray(inputs["Wk"], np.float32), np.asarray(inputs["bk"], np.float32)
    Wv, bv = np.asarray(inputs["Wv"], np.float32), np.asarray(inputs["bv"], np.float32)

    shared = {
        "wq": _prep_w_qk(Wq),
        "wk": _prep_w_qk(Wk),
        "wv": _prep_w_v(Wv),
        # sign-folded for the ACT Q evacuation: lower rows hold -bq_i so
        # out = ps*(-1) + (-bq_i) = -(Qi_raw + bq_i)
        "bq_stk": _stk(bq[:, 0], -bq[:, 1]),
        "bk_stk": _stk(bk[:, 0], bk[:, 1]),
        "bv_int": _inter(bv[:, 0] - bv[:, 1], bv[:, 0] + bv[:, 1]),
        "gam_int": _inter(
            np.asarray(inputs["gamma_r"], np.float32),
            np.asarray(inputs["gamma_i"], np.float32),
        ),
        "bet_int": _inter(
            np.asarray(inputs["beta_r"], np.float32),
            np.asarray(inputs["beta_i"], np.float32),
        ),
    }
    def _xT_full(x):
        """x [rows, D, 2] f32 -> [128, NCH, rows] fp8, ch = c*4 + d_blk."""
        a = x.astype(F8NP)            # [rows, D, 2]
        a = a.transpose(1, 2, 0)      # [D, 2, rows]
        a = a.reshape(4, 128, 2, -1)  # [db, p, c, rows]
        a = a.transpose(1, 2, 0, 3)   # [p, c, db, rows]
        return a.reshape(128, NCH, -1)

    kb, vb = {}, {}
    for b_ in range(B):
        kb[b_] = np.ascontiguousarray(
            _xT_full(k[b_]).reshape(128, NCH, 2, 512).transpose(0, 2, 1, 3)
        )
        vb[b_] = np.ascontiguousarray(
            _xT_full(v[b_]).reshape(128, NCH, NKT, 128).transpose(0, 2, 1, 3)
        )

    in_maps = []
    for c in range(NCORES):
        b_, qh = c // 2, c % 2
        qsl = slice(qh * TQ, (qh + 1) * TQ)
        in_maps.append(
            {
                "xqt": np.ascontiguousarray(
                    _xT_full(q[b_, qsl]).reshape(128, NCH * TQ)
                ),
                "xkt": kb[b_], "xvt": vb[b_],
                "rq_r": np.ascontiguousarray(
                    q[b_, qsl, :, 0].reshape(NQS, 128, D).transpose(1, 0, 2)
                ).astype(ml_dtypes.bfloat16),
                "rq_i": np.ascontiguousarray(
                    q[b_, qsl, :, 1].reshape(NQS, 128, D).transpose(1, 0, 2)
                ).astype(ml_dtypes.bfloat16),
                **shared,
            }
        )
    return in_maps


_NC_CACHE = {}
LAST_RESULT = [None]


def kernel(**inputs) -> np.ndarray:
    in_maps = host_inputs(inputs)
    ln_affine = not (
        np.all(np.asarray(inputs["gamma_r"], np.float32) == 1.0)
        and np.all(np.asarray(inputs["gamma_i"], np.float32) == 1.0)
        and np.all(np.asarray(inputs["beta_r"], np.float32) == 0.0)
        and np.all(np.asarray(inputs["beta_i"], np.float32) == 0.0)
    )
    key = ("nc", ln_affine)
    if key not in _NC_CACHE:
        _NC_CACHE[key] = build_nc(ln_affine)
    nc = _NC_CACHE[key]
    res = run_bass_kernel_spmd(nc, in_maps, list(range(NCORES)))
    LAST_RESULT[0] = res
    out = np.empty((B, S, D, 2), np.float32)
    for c in range(NCORES):
        b_, qh = c // 2, c % 2
        out[b_, qh * TQ : (qh + 1) * TQ] = np.asarray(
            res.results[c]["out"], dtype=np.float32
        )
    return out
